# revision 23
# baseline (speedup 1.0000x reference)
"""BitNet ternary layer on 8 trn2 NeuronCores — v3.

y[b,s,o] = sum_i x[b,s,i] * tq(w)[o,i],  tq(w) = sign(w) * (|w| > 0.7*mean|w|)

Distribution: data-parallel over batch — core c gets x[c] [2048,4096], the full
weight [4096,4096], and ws = weight[512c:512(c+1)] (its 1/8 slice for the
cooperative |w|-mean, gathered with an 8-rank AllGather).

Per-core pipeline (engine-queue assignment in brackets):
  A) |w| partial sums over the 8MB slice [scalar ring + DVE] -> AllGather
     [gpsimd] -> threshold t.
  B) x cast fp32->bf16 to DRAM [gpsimd SWDGE], then four 3D-batched
     DMA-transposes [sync ring] build the fully SBUF-resident
     xT [128, 32k, 2048s] bf16 (one [512,4096]->[128,32,512] xbar op per
     512-token group — batching is critical: each dma op pays ~2-4us).
  C) 8 o-chunks of 512: quantize w rows JIT (fp32 compare vs +-t; DVE, with
     the two mask temps in PSUM to save SBUF), write ternary bf16 wq to DRAM
     [scalar ring], ONE 3D transpose-load -> wqT [128, 32k, 512o] [sync],
     then 16 s-tiles x 32 k bf16 matmuls (N=512) accumulating in PSUM.
     Chunk j+1's quantize is interleaved with chunk j's matmul stream.
     PSUM->SBUF evacuation on the vector engine; y written bf16.
Host stacks the 8 per-core [2048,4096] bf16 outputs and upcasts to fp32.
"""
import copy
import sys

sys.path.insert(0, '/opt/trn_rl_repo')

import numpy as np

import concourse.bass as bass
from concourse import mybir
from concourse.tile import TileContext
from concourse.vector_clock import ScopedClock
from concourse.bass_utils import run_bass_kernel_spmd

# ---------------------------------------------------------------------------
# Workarounds for this container's walrus build rejecting sem-waits attached
# to several instruction structs (CTRL/NoOp/Drain/DMA-transpose): emit the
# TileContext exit waits as standalone wait_ge instructions, and post-process
# the module to hoist every immediate sem-wait onto its own single-wait
# InstEventSemaphore (same engine, same program position -> same semantics).
# ---------------------------------------------------------------------------


def _patched_drain_and_barrier(self, tick_clock, wait_clock):
    probe = self.nc.sync.nop()
    wait_clock.add_sem_waits(probe.ins, ScopedClock({None: tick_clock.global_clock}))
    si = probe.ins.sync_info
    waits = list(si.on_wait) if si is not None else []
    if waits:
        probe.ins.sync_info = mybir.SyncInfo(on_wait=[], on_update=list(si.on_update))
        for w in waits:
            self.nc.sync.wait_ge(bass.SemaphoreHandle(w.ant_name, w.id), w.wait_value)
    self.nc.sync.drain()
    self.nc.all_engine_barrier()
    assert self.sems is not None
    popped = self.nc._tile_sem_poison_stack.pop()
    assert popped is self._sem_poison
    self.nc.clear_and_free_semaphores(list(self.sems.allocated().values()))
    self.nc.all_engine_barrier()


TileContext._drain_and_barrier = _patched_drain_and_barrier

_ctr = [0]


def _hoist_waits(nc):
    new_module = copy.replace(nc.m, functions=[])
    for function in nc.m.functions:
        new_function = copy.replace(function, blocks=[])
        new_function.set_allocations_from_list(function.allocations)
        for block in function.blocks:
            new_insts = []
            for inst in block.instructions:
                si = inst.sync_info
                if si is not None and not isinstance(inst, mybir.InstEventSemaphore):
                    imm = [w for w in si.on_wait if w.wait_reg is None]
                    if imm:
                        reg = [w for w in si.on_wait if w.wait_reg is not None]
                        for w in imm:
                            _ctr[0] += 1
                            ev = mybir.InstEventSemaphore(
                                name=f"HW-{_ctr[0]}", ins=[], outs=[])
                            ev.engine = inst.engine
                            ev.sync_info = mybir.SyncInfo(on_wait=[w], on_update=[])
                            new_insts.append(ev)
                        inst.sync_info = mybir.SyncInfo(
                            on_wait=reg, on_update=list(si.on_update))
                new_insts.append(inst)
            new_block = copy.replace(block, instructions=new_insts)
            new_function.blocks.append(new_block)
        new_module.functions.append(new_function)
    nc.m = new_module
    return nc


# ---------------------------------------------------------------------------
# Problem shapes (hardcoded per spec)
# ---------------------------------------------------------------------------
B = 8            # batch -> one per core
S = 2048         # tokens per core
I = 4096         # in features (contraction)
O = 4096         # out features
P = 128
NK = I // P      # 32 k-tiles
OC = 512         # o-chunk width (one PSUM bank at fp32)
NOC = O // OC    # 8
OS = O // 8      # per-core weight slice rows for the cooperative absmean
NS = S // P      # 16 s-tiles
SG = 512         # x-transpose s-group
QF = 512         # quantize free-dim chunk
NQC = (OC // P) * (I // QF)   # quant chunks per o-chunk (16)


def build_program():
    fp32 = mybir.dt.float32
    bf16 = mybir.dt.bfloat16

    nc = bass.Bass()
    x_in = nc.declare_dram_parameter("x", [S, I], fp32, isOutput=False)
    w_in = nc.declare_dram_parameter("w", [O, I], fp32, isOutput=False)
    ws_in = nc.declare_dram_parameter("ws", [OS, I], fp32, isOutput=False)
    y_out = nc.declare_dram_parameter("y", [S, O], bf16, isOutput=True)

    ag_in = nc.dram_tensor("ag_in", [1, 128], fp32)
    ag_out = nc.dram_tensor("ag_out", [8, 128], fp32, addr_space="Shared")

    with TileContext(nc) as tc:
        with (
            tc.tile_pool(name="dram", bufs=1, space="DRAM") as dram,
            tc.tile_pool(name="singles", bufs=1) as singles,
            tc.tile_pool(name="psum", bufs=6, space="PSUM") as psum_pool,
            tc.tile_pool(name="qpsum", bufs=1, space="PSUM") as qpsum,
            tc.tile_pool(name="xT_pool", bufs=1) as xT_pool,
            tc.tile_pool(name="wqT_pool", bufs=2) as wqT_pool,
            tc.tile_pool(name="quant", bufs=2) as quant,
            tc.tile_pool(name="outsb", bufs=3) as outsb,
        ):
            x16 = dram.tile([S, I], bf16)
            wq_oc = [dram.tile([2, OC, I // 2], bf16, name=f"wq{oc}")
                     for oc in range(NOC)]
            t_dram = dram.tile([1, 1], fp32)

            partials = singles.tile([P, 4], fp32)
            part1 = singles.tile([P, 1], fp32)
            ts8 = singles.tile([8, 128], fp32)
            ts81 = singles.tile([8, 1], fp32)
            ones8 = singles.tile([8, 1], fp32)
            ones128 = singles.tile([1, P], fp32)
            tval = singles.tile([1, 1], fp32)
            t_b = singles.tile([P, 1], fp32)
            nt_b = singles.tile([P, 1], fp32)

            # ---- Phase A: local |w| partials over the 8MB slice ----
            # (reads borrow the big wqT pool slots, which are idle here)
            for j in range(4):
                wa = wqT_pool.tile([P, I], fp32, tag="wqT", bufs=2)
                nc.scalar.dma_start(
                    out=wa[:], in_=ws_in[j * P:(j + 1) * P, :])
                nc.vector.tensor_reduce(
                    partials[:, j:j + 1], wa[:],
                    axis=mybir.AxisListType.X,
                    op=mybir.AluOpType.add,
                    apply_absolute_value=True)
            nc.vector.tensor_reduce(
                part1[:], partials[:], axis=mybir.AxisListType.X,
                op=mybir.AluOpType.add)
            # ---- AllGather first on the gpsimd ring: ag_in is the FIRST
            #      DMA on its lane so the collective's hoisted lane-wait
            #      counts exactly this write (a later position lets two cast
            #      completions satisfy the wait before ag_in lands -- a
            #      nondeterministic threshold race). The trigger itself is
            #      fire-and-forget, so the casts behind it start at ~27us. ----
            nc.gpsimd.dma_start(out=ag_in[:], in_=part1[:])
            nc.gpsimd.collective_compute(
                "AllGather",
                mybir.AluOpType.bypass,
                replica_groups=[[0, 1, 2, 3, 4, 5, 6, 7]],
                ins=[ag_in[:]],
                outs=[ag_out[:]],
            )

            # ---- Phase B: cast x fp32 -> bf16 in DRAM (SWDGE cast) ----
            for j in range(S // P):
                nc.gpsimd.dma_start(
                    out=x16[j * P:(j + 1) * P, :],
                    in_=x_in[j * P:(j + 1) * P, :])

            # readback on the sync ring: gated by the Collectives-lane
            # semaphore (fires at completion), not behind the cast queue
            nc.sync.dma_start(out=ts8[:], in_=ag_out[:])
            nc.vector.tensor_reduce(
                ts81[:], ts8[:], axis=mybir.AxisListType.X,
                op=mybir.AluOpType.add)
            nc.vector.memset(ones8[:], 1.0)
            tsp = qpsum.tile([1, 1], fp32, tag="tsp")
            nc.tensor.matmul(tsp[:], lhsT=ts81[:], rhs=ones8[:],
                             start=True, stop=True)
            nc.scalar.activation(tval[:], tsp[:],
                                 mybir.ActivationFunctionType.Copy,
                                 scale=0.7 / float(O * I))
            nc.sync.dma_start(out=t_dram[:], in_=tval[:])
            t_bcast_ap = bass.AP(
                tensor=t_dram.tensor, offset=t_dram.offset,
                ap=[[0, P], [1, 1]])
            nc.sync.dma_start(out=t_b[:], in_=t_bcast_ap)
            nc.vector.tensor_scalar_mul(nt_b[:], t_b[:], -1.0)

            # ---- xT: 3D-batched transpose loads (sync ring) ----
            # sg-major layout keeps each group's written region disjoint so
            # Tile's AP-overlap tracking doesn't serialize matmuls on
            # unrelated transpose groups.
            xT = xT_pool.tile([P, S // SG, NK, SG], bf16)

            def xT_group(sg):
                # [512, 4096] -> [128, 32, 512]: xT[p,sg,k,s'] = x16[s, k*128+p]
                nc.sync.dma_start(
                    out=xT[:, sg, :, :],
                    in_=x16[sg * SG:(sg + 1) * SG, :],
                    transpose=True)

            # ---- quantize one o-chunk (chunks of [128, QF]) ----
            pt_t = singles.tile([P, QF], bf16)
            nt_t = singles.tile([P, QF], bf16)

            def quant_chunks(oc, lo, hi):
                # i-half-major order: all of half 0's chunks first, so the
                # first wqT half-transpose can fire ~35us before the second
                for ci in range(lo, hi):
                    ih, sub = divmod(ci, NQC // 2)
                    cc4, rb = divmod(sub, OC // P)
                    col = ih * (I // 2) + cc4 * QF
                    wb = quant.tile([P, QF], fp32, tag="wb")
                    nc.scalar.dma_start(
                        out=wb[:],
                        in_=w_in[oc * OC + rb * P:oc * OC + (rb + 1) * P,
                                 col:col + QF])
                    qt = quant.tile([P, QF], bf16, tag="qt")
                    nc.vector.tensor_scalar(
                        pt_t[:], wb[:], t_b[:], None,
                        op0=mybir.AluOpType.is_gt)
                    nc.vector.tensor_scalar(
                        nt_t[:], wb[:], nt_b[:], None,
                        op0=mybir.AluOpType.is_lt)
                    nc.vector.tensor_sub(qt[:], pt_t[:], nt_t[:])
                    nc.gpsimd.dma_start(
                        out=wq_oc[oc][ih, rb * P:(rb + 1) * P,
                                      cc4 * QF:(cc4 + 1) * QF],
                        in_=qt[:])

            def wqT_load(oc, wqT):
                # per i-half: [512, 2048] -> [128, 16, 512];
                # wqT[p, ih*16+k', o] = wq[o, ih*2048 + k'*128 + p]
                for ih in range(2):
                    nc.sync.dma_start(
                        out=wqT[:, ih * 16:(ih + 1) * 16, :],
                        in_=wq_oc[oc][ih, :, :],
                        transpose=True)

            # prologue: x groups 0/1, then chunk 0, then x groups 2/3
            xT_group(0)
            xT_group(1)
            quant_chunks(0, 0, NQC)
            wqT_cur = wqT_pool.tile([P, NK, OC], bf16, tag="wqT")
            wqT_load(0, wqT_cur)
            xT_group(2)
            xT_group(3)

            # ---- Phase C: matmul stream with 1-chunk lookahead ----
            for oc in range(NOC):
                # next chunk's quantize: reads (scalar ring) are dep-free and
                # DVE-paced, so the whole block drains in ~35us regardless of
                # the matmul stream's progress
                if oc + 1 < NOC:
                    quant_chunks(oc + 1, 0, NQC)
                    wqT_next = wqT_pool.tile([P, NK, OC], bf16, tag="wqT")
                    wqT_load(oc + 1, wqT_next)
                else:
                    wqT_next = None
                for s in range(NS):
                    ps = psum_pool.tile([P, OC], fp32, tag="ps")
                    sg, ss = divmod(s, SG // P)
                    for k in range(NK):
                        nc.tensor.matmul(
                            ps[:],
                            lhsT=xT[:, sg, k, ss * P:(ss + 1) * P],
                            rhs=wqT_cur[:, k, :],
                            start=(k == 0),
                            stop=(k == NK - 1))
                    ob = outsb.tile([P, OC], bf16, tag="ob")
                    nc.vector.tensor_copy(ob[:], ps[:])
                    nc.gpsimd.dma_start(
                        out=y_out[s * P:(s + 1) * P, oc * OC:(oc + 1) * OC],
                        in_=ob[:])
                if wqT_next is not None:
                    wqT_cur = wqT_next

    _hoist_waits(nc)
    return nc


_program_cache = None


def _get_program():
    global _program_cache
    if _program_cache is None:
        _program_cache = build_program()
    return _program_cache


def run(x, weight, trace=False):
    x = np.asarray(x, dtype=np.float32)
    weight = np.ascontiguousarray(np.asarray(weight, dtype=np.float32))
    assert x.shape == (B, S, I), x.shape
    assert weight.shape == (O, I), weight.shape
    nc = _get_program()
    in_maps = [
        {
            "x": np.ascontiguousarray(x[c]),
            "w": weight,
            "ws": np.ascontiguousarray(weight[c * OS:(c + 1) * OS]),
        }
        for c in range(B)
    ]
    res = run_bass_kernel_spmd(nc, in_maps, list(range(B)), trace=trace)
    y = np.stack(
        [np.asarray(res.results[c]["y"], dtype=np.float32) for c in range(B)],
        axis=0)
    return y, res


def kernel(x, weight):
    y, _ = run(x, weight)
    return y
